# revision 9
# baseline (speedup 1.0000x reference)
"""GCN 4-hop message passing on 8 Trainium2 NeuronCores.

Strategy:
  - Nodes are assigned to 128-wide "chunks" with degree-balanced packing (LPT);
    core m owns chunks [m*CPC, (m+1)*CPC). Edges are partitioned by destination
    chunk; within a (chunk, src-half) segment they are padded to a fixed
    number K of 128-edge blocks so the SPMD program is identical on all cores.
  - Per hop: each core dma_gathers source rows (channel-interleaved bf16 table
    in HBM, two halves for int16 indices; <=1024 idx per instruction so
    single_packet descriptor generation applies), loads host-precomputed
    one-hot-times-weight S blocks from HBM, and segment-sums via TensorEngine
    matmuls accumulated in PSUM. Node update (beta mix + norm) feeds an
    AllGather replicating the updated table to all cores.
  - Final per-graph Linear + ReLU via PE transpose + matmul.

Host-side work is limited to integer index/schedule construction, the static
one-hot weight blocks, and input/output reshuffling; all graph compute
(gather, message scaling, aggregation, update, linear) runs on device.
"""
import math

import numpy as np
import ml_dtypes

import concourse.bacc as bacc
import concourse.bass as bass
import concourse.mybir as mybir
import concourse.tile as tile
from concourse.bass_utils import run_bass_kernel_spmd

P = 128
NCORES = 8
G = 2
BETA = 0.1
NUM_HOP = 4
MAX_GATHER = 1024  # single_packet limit: 64 descs x 16 engines

F32 = mybir.dt.float32
BF16 = mybir.dt.bfloat16
I16 = mybir.dt.int16

_NC_CACHE = {}


# --------------------------------------------------------------------------
# Host preprocessing
# --------------------------------------------------------------------------

def _lpt_pack(indeg, nchunk):
    """Assign nodes to nchunk chunks of P slots, balancing degree sums.

    Returns perm: node -> global slot id."""
    import heapq

    n = indeg.shape[0]
    order = np.argsort(-indeg, kind="stable")
    heap = [(0, c) for c in range(nchunk)]
    heapq.heapify(heap)
    counts = np.zeros(nchunk, dtype=np.int64)
    perm = np.empty(n, dtype=np.int64)
    deg = indeg.astype(np.int64)
    for v in order:
        s, c = heapq.heappop(heap)
        perm[v] = c * P + counts[c]
        counts[c] += 1
        if counts[c] < P:
            heapq.heappush(heap, (s + deg[v], c))
    return perm


def _preprocess(features, src, dst, edge_factors, cpc):
    """Build per-core input arrays and the static schedule structure."""
    n, d = features.shape
    assert d == P
    nchunk = NCORES * cpc
    npad = nchunk * P
    npc = cpc * P
    half = npad // 2
    assert half <= 32768, f"half {half} exceeds int16 range"

    indeg = np.bincount(dst, minlength=n).astype(np.int64)
    norm = 1.0 / np.sqrt(np.clip(indeg, 1, None).astype(np.float64))
    perm = _lpt_pack(indeg, nchunk)

    feat_slot = np.zeros((npad, d), dtype=np.float32)
    feat_slot[perm] = np.asarray(features, dtype=np.float32)
    norm_slot = np.ones(npad, dtype=np.float32)
    norm_slot[perm] = norm.astype(np.float32)

    dslot = perm[dst]
    sslot = perm[src]
    ecore = dslot // npc
    ef0 = np.asarray(edge_factors[0], dtype=np.float32)
    ef1 = np.asarray(edge_factors[1], dtype=np.float32)

    per_core = []
    kmax = 1
    for m in range(NCORES):
        sel = np.nonzero(ecore == m)[0]
        ds = dslot[sel] % npc
        ch = ds // P
        dl = (ds % P).astype(np.int64)
        hf = sslot[sel] // half
        sx = (sslot[sel] % half).astype(np.int64)
        seg = hf * cpc + ch  # stream-major: half, then chunk
        o2 = np.lexsort((sx, seg))
        seg, sx, dl = seg[o2], sx[o2], dl[o2]
        w0, w1 = ef0[sel][o2], ef1[sel][o2]
        cnt = np.bincount(seg, minlength=cpc * 2)
        kmax = max(kmax, int(math.ceil(cnt.max() / P)))
        per_core.append((seg, sx, dl, w0, w1, cnt))

    K = kmax
    btot = cpc * 2 * K
    # block id of (c, h, k) = (h*cpc + c)*K + k
    # gather instruction pieces: within each half-stream, runs of <= 8 blocks
    blocks_per_half = cpc * K
    pieces = []  # (block0, nblk, half)
    maxb = MAX_GATHER // P
    for h in (0, 1):
        b = h * blocks_per_half
        end = (h + 1) * blocks_per_half
        while b < end:
            nb = min(maxb, end - b)
            pieces.append((b, nb, h))
            b += nb

    in_maps = []
    ident = np.eye(P, dtype=ml_dtypes.bfloat16)

    for m in range(NCORES):
        seg, sx, dl, w0, w1, cnt = per_core[m]
        starts = np.zeros(cpc * 2, dtype=np.int64)
        starts[1:] = np.cumsum(cnt)[:-1]

        s_idx = np.zeros(btot * P, dtype=np.int64)
        s_dl = np.zeros(btot * P, dtype=np.int64)
        s_w0 = np.zeros(btot * P, dtype=np.float32)
        s_w1 = np.zeros(btot * P, dtype=np.float32)
        for s in range(cpc * 2):
            # seg s = hf*cpc + ch maps to block base s*K
            c0 = s * K * P
            k = int(cnt[s])
            st = starts[s]
            s_idx[c0:c0 + k] = sx[st:st + k]
            s_dl[c0:c0 + k] = dl[st:st + k]
            s_w0[c0:c0 + k] = w0[st:st + k]
            s_w1[c0:c0 + k] = w1[st:st + k]

        idx_all = np.zeros((128, btot * 8), dtype=np.int16)
        for (b0, nblk, _h) in pieces:
            v = s_idx[b0 * P:(b0 + nblk) * P].astype(np.int16)
            idx_all[:16, b0 * 8:(b0 + nblk) * 8] = v.reshape(nblk * 8, 16).T
        idx_all[16:] = np.tile(idx_all[:16], (7, 1))

        # S blocks: [128 partitions(edge slot in block), btot, 256]
        sh = np.zeros((128, btot, 2 * P), dtype=ml_dtypes.bfloat16)
        eb = np.arange(btot * P) // P
        ep = np.arange(btot * P) % P
        sh[ep, eb, s_dl] = s_w0.astype(ml_dtypes.bfloat16)
        sh[ep, eb, P + s_dl] = s_w1.astype(ml_dtypes.bfloat16)

        in_maps.append({
            "feat": feat_slot[m * npc:(m + 1) * npc],
            "normc": norm_slot[m * npc:(m + 1) * npc].reshape(cpc, P).T.copy(),
            "idx_all": idx_all,
            "sblk": sh.reshape(128, btot * 2 * P),
            "ident": ident,
        })

    struct = dict(cpc=cpc, K=K, pieces=pieces,
                  npad=npad, npc=npc, half=half)
    return in_maps, struct, perm


# --------------------------------------------------------------------------
# Bass program
# --------------------------------------------------------------------------

def _build(struct):
    cpc = struct["cpc"]
    K = struct["K"]
    pieces = struct["pieces"]
    npad = struct["npad"]
    npc = struct["npc"]
    half = struct["half"]
    D = P
    D2 = 2 * P
    btot = cpc * 2 * K

    # block id -> (piece index, col within piece)
    blk_piece = {}
    for pi, (b0, nblk, _h) in enumerate(pieces):
        for j in range(nblk):
            blk_piece[b0 + j] = (pi, j)

    nc = bacc.Bacc("TRN2", target_bir_lowering=False, debug=False,
                   enable_asserts=False, num_devices=NCORES)

    feat = nc.dram_tensor("feat", [npc, D], F32, kind="ExternalInput").ap()
    normc_d = nc.dram_tensor("normc", [P, cpc], F32, kind="ExternalInput").ap()
    idx_d = nc.dram_tensor("idx_all", [128, btot * 8], I16, kind="ExternalInput").ap()
    sblk_d = nc.dram_tensor("sblk", [128, btot * D2], BF16, kind="ExternalInput").ap()
    ident_d = nc.dram_tensor("ident", [P, P], BF16, kind="ExternalInput").ap()
    W_d = nc.dram_tensor("W_in", [P, D2], F32, kind="ExternalInput").ap()
    b_d = nc.dram_tensor("b_repl", [P, D2], F32, kind="ExternalInput").ap()
    out = nc.dram_tensor("out", [npc, D2], F32, kind="ExternalOutput").ap()

    AGOP = mybir.AluOpType.bypass
    ADD = mybir.AluOpType.add
    MUL = mybir.AluOpType.mult
    MAX = mybir.AluOpType.max

    with tile.TileContext(nc) as tc:
        with (
            tc.tile_pool(name="const", bufs=1) as cp,
            tc.tile_pool(name="state", bufs=1) as sp,
            tc.tile_pool(name="msg", bufs=10) as mp,
            tc.tile_pool(name="sload", bufs=4) as slp,
            tc.tile_pool(name="work", bufs=3) as wp,
            tc.tile_pool(name="psum", bufs=4, space="PSUM") as pp,
            tc.tile_pool(name="dram", bufs=1, space="DRAM") as dp,
        ):
            normc = cp.tile([P, cpc], F32, tag="normc")
            idx_all = cp.tile([128, btot * 8], I16, tag="idx")
            ident = cp.tile([P, P], BF16, tag="ident")
            Wt = cp.tile([P, D2], F32, tag="W")
            bt = cp.tile([P, D2], F32, tag="b")
            h0b = sp.tile([P, cpc, D], BF16, tag="h0b")
            hcur = sp.tile([P, cpc, D2], BF16, tag="hcur")

            for t_, d_ in ((normc, normc_d), (idx_all, idx_d),
                           (ident, ident_d), (Wt, W_d), (bt, b_d)):
                nc.sync.dma_start(t_[:], d_[:])

            tables = [dp.tile([npad, D2], BF16, tag=f"table{t}", name=f"table{t}")
                      for t in range(NUM_HOP)]
            agin = [dp.tile([npc, D2], BF16, tag=f"agin{i}", name=f"agin{i}")
                    for i in range(2)]

            # ---- prologue: table0 = (features * norm) duplicated per channel
            for c in range(cpc):
                ft = wp.tile([P, D], F32, tag="ft")
                nc.sync.dma_start(ft[:], feat[c * P:(c + 1) * P, :])
                nc.vector.tensor_scalar(out=h0b[:, c, :], in0=ft[:],
                                        scalar1=BETA, scalar2=None, op0=MUL)
                hp = wp.tile([P, D2], BF16, tag="hp")
                nc.vector.tensor_scalar(out=hp[:, 0:D], in0=ft[:],
                                        scalar1=normc[:, c:c + 1], scalar2=None,
                                        op0=MUL)
                nc.vector.tensor_scalar(out=hp[:, D:D2], in0=ft[:],
                                        scalar1=normc[:, c:c + 1], scalar2=None,
                                        op0=MUL)
                nc.sync.dma_start(agin[0][c * P:(c + 1) * P, :], hp[:])
            nc.gpsimd.collective_compute(
                "AllGather", AGOP, replica_groups=[list(range(NCORES))],
                ins=[agin[0].opt()], outs=[tables[0].opt()])

            # ---- hops
            for t in range(NUM_HOP):
                tbl = tables[t]
                halves = (tbl[0:half, :], tbl[half:npad, :])
                ptiles = [None] * len(pieces)

                def emit_piece(pi, ptiles=ptiles, halves=halves, t=t):
                    if ptiles[pi] is not None:
                        return
                    b0, nblk, h = pieces[pi]
                    mt = mp.tile([P, nblk, D2], BF16, tag="msg",
                                 name=f"msg_t{t}_p{pi}")
                    nc.gpsimd.dma_gather(
                        mt[:], halves[h],
                        idx_all[:, b0 * 8:(b0 + nblk) * 8],
                        nblk * P, nblk * P, D2, single_packet=True)
                    ptiles[pi] = mt

                for c in range(cpc):
                    sts = []
                    for h in (0, 1):
                        b0 = (h * cpc + c) * K
                        st = slp.tile([P, K, D2], BF16, tag="sblk",
                                      name=f"sblk_t{t}_c{c}_h{h}")
                        nc.sync.dma_start(
                            st[:], sblk_d[:, b0 * D2:(b0 + K) * D2])
                        sts.append(st)
                        for k in range(K):
                            emit_piece(blk_piece[b0 + k][0])
                    ps = pp.tile([P, D2], F32, tag="agg", space="PSUM", bufs=3)
                    nmm = 2 * K
                    for ch in (0, 1):
                        mi = 0
                        for h in (0, 1):
                            b0 = (h * cpc + c) * K
                            for k in range(K):
                                pi, col = blk_piece[b0 + k]
                                mt = ptiles[pi]
                                nc.tensor.matmul(
                                    out=ps[:, ch * D:(ch + 1) * D],
                                    lhsT=sts[h][:, k, ch * D:(ch + 1) * D],
                                    rhs=mt[:, col, ch * D:(ch + 1) * D],
                                    start=(mi == 0),
                                    stop=(mi == nmm - 1))
                                mi += 1
                    tsb = wp.tile([P, D2], F32, tag="tsb")
                    nc.scalar.mul(tsb[:], ps[:], 1.0 - BETA)
                    nc.vector.tensor_tensor(out=hcur[:, c, 0:D],
                                            in0=tsb[:, 0:D],
                                            in1=h0b[:, c, :], op=ADD)
                    nc.vector.tensor_tensor(out=hcur[:, c, D:D2],
                                            in0=tsb[:, D:D2],
                                            in1=h0b[:, c, :], op=ADD)
                    if t < NUM_HOP - 1:
                        hp = wp.tile([P, D2], BF16, tag="hp")
                        nc.vector.tensor_scalar(
                            out=hp[:], in0=hcur[:, c, :],
                            scalar1=normc[:, c:c + 1], scalar2=None, op0=MUL)
                        nc.sync.dma_start(
                            agin[(t + 1) % 2][c * P:(c + 1) * P, :], hp[:])
                if t < NUM_HOP - 1:
                    nc.gpsimd.collective_compute(
                        "AllGather", AGOP, replica_groups=[list(range(NCORES))],
                        ins=[agin[(t + 1) % 2].opt()],
                        outs=[tables[t + 1].opt()])

            # ---- final linear + relu
            for c in range(cpc):
                po = pp.tile([P, D2], F32, tag="pout", space="PSUM", bufs=2)
                for ch in (0, 1):
                    tp = pp.tile([P, P], BF16, tag="tps", space="PSUM", bufs=2)
                    nc.tensor.transpose(out=tp[:],
                                        in_=hcur[:, c, ch * D:(ch + 1) * D],
                                        identity=ident[:])
                    h4t = wp.tile([P, P], F32, tag="h4t")
                    nc.scalar.copy(h4t[:], tp[:])
                    nc.tensor.matmul(out=po[:, ch * D:(ch + 1) * D], lhsT=h4t[:],
                                     rhs=Wt[:, ch * D:(ch + 1) * D],
                                     start=True, stop=True)
                ob = wp.tile([P, D2], F32, tag="ob")
                nc.vector.tensor_tensor(out=ob[:], in0=po[:], in1=bt[:], op=ADD)
                ob2 = wp.tile([P, D2], F32, tag="ob2")
                nc.vector.tensor_scalar(out=ob2[:], in0=ob[:], scalar1=0.0,
                                        scalar2=None, op0=MAX)
                nc.sync.dma_start(out[c * P:(c + 1) * P, :], ob2[:])

    nc.compile()
    return nc


# --------------------------------------------------------------------------
# Entry point
# --------------------------------------------------------------------------

def run(features, src, dst, edge_factors, W, b, cpc=49, trace=False):
    features = np.asarray(features, dtype=np.float32)
    src = np.asarray(src, dtype=np.int32)
    dst = np.asarray(dst, dtype=np.int32)
    edge_factors = np.asarray(edge_factors, dtype=np.float32)
    W = np.asarray(W, dtype=np.float32)
    b = np.asarray(b, dtype=np.float32)

    in_maps, struct, perm = _preprocess(features, src, dst, edge_factors, cpc)
    W_in = np.concatenate([W[0], W[1]], axis=1).astype(np.float32)
    b_repl = np.tile(np.concatenate([b[0], b[1]])[None, :], (P, 1)).astype(np.float32)
    for im in in_maps:
        im["W_in"] = W_in
        im["b_repl"] = b_repl

    key = (struct["cpc"], struct["K"])
    nc = _NC_CACHE.get(key)
    if nc is None:
        nc = _build(struct)
        _NC_CACHE[key] = nc

    res = run_bass_kernel_spmd(nc, in_maps, core_ids=list(range(NCORES)),
                               trace=trace)
    out_all = np.concatenate([res.results[m]["out"] for m in range(NCORES)], axis=0)
    result = out_all[perm]  # perm maps node -> slot
    return result.astype(np.float32), res


def kernel(**inputs):
    result, _ = run(**inputs)
    return result


# revision 11
# speedup vs baseline: 1.3689x; 1.3689x over previous
"""GCN 4-hop message passing on 8 Trainium2 NeuronCores.

Strategy:
  - Nodes are assigned to 128-wide "chunks" with degree-balanced packing (LPT);
    core m owns chunks [m*CPC, (m+1)*CPC). Edges are partitioned by destination
    chunk; within a (chunk, src-half) segment they are padded to a fixed
    number K of 128-edge blocks so the SPMD program is identical on all cores.
  - Per hop: each core dma_gathers source rows (channel-interleaved bf16 table
    in HBM, two halves for int16 indices; <=1024 idx per instruction so
    single_packet descriptor generation applies), loads host-precomputed
    one-hot-times-weight S blocks from HBM, and segment-sums via TensorEngine
    matmuls accumulated in PSUM. Node update (beta mix + norm) feeds an
    AllGather replicating the updated table to all cores.
  - Final per-graph Linear + ReLU via PE transpose + matmul.

Host-side work is limited to integer index/schedule construction, the static
one-hot weight blocks, and input/output reshuffling; all graph compute
(gather, message scaling, aggregation, update, linear) runs on device.
"""
import math

import numpy as np
import ml_dtypes

import concourse.bacc as bacc
import concourse.bass as bass
import concourse.mybir as mybir
import concourse.tile as tile
from concourse.bass_utils import run_bass_kernel_spmd

P = 128
NCORES = 8
G = 2
BETA = 0.1
NUM_HOP = 4
MAX_GATHER = 1024  # single_packet limit: 64 descs x 16 engines
NQUEUES = 4  # parallel SWDGE descriptor-generation queues

F32 = mybir.dt.float32
BF16 = mybir.dt.bfloat16
I16 = mybir.dt.int16

_NC_CACHE = {}


# --------------------------------------------------------------------------
# Host preprocessing
# --------------------------------------------------------------------------

def _lpt_pack(indeg, nchunk):
    """Assign nodes to nchunk chunks of P slots, balancing degree sums.

    Returns perm: node -> global slot id."""
    import heapq

    n = indeg.shape[0]
    order = np.argsort(-indeg, kind="stable")
    heap = [(0, c) for c in range(nchunk)]
    heapq.heapify(heap)
    counts = np.zeros(nchunk, dtype=np.int64)
    perm = np.empty(n, dtype=np.int64)
    deg = indeg.astype(np.int64)
    for v in order:
        s, c = heapq.heappop(heap)
        perm[v] = c * P + counts[c]
        counts[c] += 1
        if counts[c] < P:
            heapq.heappush(heap, (s + deg[v], c))
    return perm


def _preprocess(features, src, dst, edge_factors, cpc, nsplit):
    """Build per-core input arrays and the static schedule structure."""
    n, d = features.shape
    assert d == P
    assert cpc % nsplit == 0
    cpg = cpc // nsplit          # chunks per split region (per core)
    nchunk = NCORES * cpc
    npad = nchunk * P
    npc = cpc * P
    rs = NCORES * cpg * P        # rows per split region of the table
    half = npad // 2
    assert half <= 32768, f"half {half} exceeds int16 range"

    indeg = np.bincount(dst, minlength=n).astype(np.int64)
    norm = 1.0 / np.sqrt(np.clip(indeg, 1, None).astype(np.float64))
    perm = _lpt_pack(indeg, nchunk)

    # decompose LPT slot into (core m, position pos, lane i)
    cg = perm // P
    lane = perm % P
    m_of = cg // cpc
    pos_of = cg % cpc
    # table row (split-region-major, rank-major inside region: AllGather layout)
    perm_row = ((pos_of // cpg) * rs + m_of * (cpg * P)
                + (pos_of % cpg) * P + lane)
    # output index (core-major, position-major)
    perm_out = m_of * npc + pos_of * P + lane

    feat_slot = np.zeros((npad, d), dtype=np.float32)
    feat_slot[perm_out] = np.asarray(features, dtype=np.float32)
    norm_slot = np.ones(npad, dtype=np.float32)
    norm_slot[perm_out] = norm.astype(np.float32)

    e_m = m_of[dst]
    e_pos = pos_of[dst]
    e_dl = lane[dst]
    srow = perm_row[src]
    ef0 = np.asarray(edge_factors[0], dtype=np.float32)
    ef1 = np.asarray(edge_factors[1], dtype=np.float32)

    per_core = []
    kmax = 1
    for m in range(NCORES):
        sel = np.nonzero(e_m == m)[0]
        ch = e_pos[sel]
        dl = e_dl[sel].astype(np.int64)
        hf = srow[sel] // half
        sx = (srow[sel] % half).astype(np.int64)
        seg = hf * cpc + ch  # stream-major: half, then chunk position
        o2 = np.lexsort((sx, seg))
        seg, sx, dl = seg[o2], sx[o2], dl[o2]
        w0, w1 = ef0[sel][o2], ef1[sel][o2]
        cnt = np.bincount(seg, minlength=cpc * 2)
        kmax = max(kmax, int(math.ceil(cnt.max() / P)))
        per_core.append((seg, sx, dl, w0, w1, cnt))

    K = kmax
    btot = cpc * 2 * K
    # block id of (c, h, k) = (h*cpc + c)*K + k
    # gather instruction pieces: within each half-stream, runs of <= 8 blocks
    blocks_per_half = cpc * K
    pieces = []  # (block0, nblk, half)
    maxb = MAX_GATHER // P
    for h in (0, 1):
        b = h * blocks_per_half
        end = (h + 1) * blocks_per_half
        while b < end:
            nb = min(maxb, end - b)
            pieces.append((b, nb, h))
            b += nb

    in_maps = []
    ident = np.eye(P, dtype=ml_dtypes.bfloat16)

    for m in range(NCORES):
        seg, sx, dl, w0, w1, cnt = per_core[m]
        starts = np.zeros(cpc * 2, dtype=np.int64)
        starts[1:] = np.cumsum(cnt)[:-1]

        s_idx = np.zeros(btot * P, dtype=np.int64)
        s_dl = np.zeros(btot * P, dtype=np.int64)
        s_w0 = np.zeros(btot * P, dtype=np.float32)
        s_w1 = np.zeros(btot * P, dtype=np.float32)
        for s in range(cpc * 2):
            # seg s = hf*cpc + ch maps to block base s*K
            c0 = s * K * P
            k = int(cnt[s])
            st = starts[s]
            s_idx[c0:c0 + k] = sx[st:st + k]
            s_dl[c0:c0 + k] = dl[st:st + k]
            s_w0[c0:c0 + k] = w0[st:st + k]
            s_w1[c0:c0 + k] = w1[st:st + k]

        idx_all = np.zeros((128, btot * 8), dtype=np.int16)
        for (b0, nblk, _h) in pieces:
            v = s_idx[b0 * P:(b0 + nblk) * P].astype(np.int16)
            idx_all[:16, b0 * 8:(b0 + nblk) * 8] = v.reshape(nblk * 8, 16).T
        idx_all[16:] = np.tile(idx_all[:16], (7, 1))

        # S blocks: [128 partitions(edge slot in block), btot, 256]
        sh = np.zeros((128, btot, 2 * P), dtype=ml_dtypes.bfloat16)
        eb = np.arange(btot * P) // P
        ep = np.arange(btot * P) % P
        sh[ep, eb, s_dl] = s_w0.astype(ml_dtypes.bfloat16)
        sh[ep, eb, P + s_dl] = s_w1.astype(ml_dtypes.bfloat16)

        in_maps.append({
            "feat": feat_slot[m * npc:(m + 1) * npc],
            "normc": norm_slot[m * npc:(m + 1) * npc].reshape(cpc, P).T.copy(),
            "idx_all": idx_all,
            "sblk": sh.reshape(128, btot * 2 * P),
            "ident": ident,
        })

    struct = dict(cpc=cpc, K=K, pieces=pieces, nsplit=nsplit, cpg=cpg, rs=rs,
                  npad=npad, npc=npc, half=half)
    return in_maps, struct, perm_out


# --------------------------------------------------------------------------
# Bass program
# --------------------------------------------------------------------------

def _build(struct):
    cpc = struct["cpc"]
    K = struct["K"]
    pieces = struct["pieces"]
    npad = struct["npad"]
    npc = struct["npc"]
    half = struct["half"]
    nsplit = struct["nsplit"]
    cpg = struct["cpg"]
    rs = struct["rs"]
    D = P
    D2 = 2 * P
    btot = cpc * 2 * K

    # block id -> (piece index, col within piece)
    blk_piece = {}
    for pi, (b0, nblk, _h) in enumerate(pieces):
        for j in range(nblk):
            blk_piece[b0 + j] = (pi, j)

    nc = bacc.Bacc("TRN2", target_bir_lowering=False, debug=False,
                   enable_asserts=False, num_devices=NCORES,
                   num_swdge_queues=NQUEUES)

    feat = nc.dram_tensor("feat", [npc, D], F32, kind="ExternalInput").ap()
    normc_d = nc.dram_tensor("normc", [P, cpc], F32, kind="ExternalInput").ap()
    idx_d = nc.dram_tensor("idx_all", [128, btot * 8], I16, kind="ExternalInput").ap()
    sblk_d = nc.dram_tensor("sblk", [128, btot * D2], BF16, kind="ExternalInput").ap()
    ident_d = nc.dram_tensor("ident", [P, P], BF16, kind="ExternalInput").ap()
    W_d = nc.dram_tensor("W_in", [P, D2], F32, kind="ExternalInput").ap()
    b_d = nc.dram_tensor("b_repl", [P, D2], F32, kind="ExternalInput").ap()
    out = nc.dram_tensor("out", [npc, D2], F32, kind="ExternalOutput").ap()

    AGOP = mybir.AluOpType.bypass
    ADD = mybir.AluOpType.add
    MUL = mybir.AluOpType.mult
    MAX = mybir.AluOpType.max

    with tile.TileContext(nc) as tc:
        with (
            tc.tile_pool(name="const", bufs=1) as cp,
            tc.tile_pool(name="state", bufs=1) as sp,
            tc.tile_pool(name="msg", bufs=10) as mp,
            tc.tile_pool(name="sload", bufs=4) as slp,
            tc.tile_pool(name="work", bufs=3) as wp,
            tc.tile_pool(name="psum", bufs=4, space="PSUM") as pp,
            tc.tile_pool(name="dram", bufs=1, space="DRAM") as dp,
        ):
            normc = cp.tile([P, cpc], F32, tag="normc")
            idx_all = cp.tile([128, btot * 8], I16, tag="idx")
            ident = cp.tile([P, P], BF16, tag="ident")
            Wt = cp.tile([P, D2], F32, tag="W")
            bt = cp.tile([P, D2], F32, tag="b")
            h0b = sp.tile([P, cpc, D], BF16, tag="h0b")
            hcur = sp.tile([P, cpc, D2], BF16, tag="hcur")

            for t_, d_ in ((normc, normc_d), (idx_all, idx_d),
                           (ident, ident_d), (Wt, W_d), (bt, b_d)):
                nc.sync.dma_start(t_[:], d_[:])

            tables = [dp.tile([npad, D2], BF16, tag=f"table{t}", name=f"table{t}")
                      for t in range(NUM_HOP)]
            agin = [[dp.tile([cpg * P, D2], BF16, tag=f"agin{i}_{sl}",
                             name=f"agin{i}_{sl}") for sl in range(nsplit)]
                    for i in range(2)]

            # ---- prologue: table0 = (features * norm) duplicated per channel
            for c in range(cpc):
                ft = wp.tile([P, D], F32, tag="ft")
                nc.sync.dma_start(ft[:], feat[c * P:(c + 1) * P, :])
                nc.vector.tensor_scalar(out=h0b[:, c, :], in0=ft[:],
                                        scalar1=BETA, scalar2=None, op0=MUL)
                hp = wp.tile([P, D2], BF16, tag="hp")
                nc.vector.tensor_scalar(out=hp[:, 0:D], in0=ft[:],
                                        scalar1=normc[:, c:c + 1], scalar2=None,
                                        op0=MUL)
                nc.vector.tensor_scalar(out=hp[:, D:D2], in0=ft[:],
                                        scalar1=normc[:, c:c + 1], scalar2=None,
                                        op0=MUL)
                sl, cl = divmod(c, cpg)
                nc.sync.dma_start(agin[0][sl][cl * P:(cl + 1) * P, :], hp[:])
                if cl == cpg - 1:
                    nc.gpsimd.collective_compute(
                        "AllGather", AGOP,
                        replica_groups=[list(range(NCORES))],
                        ins=[agin[0][sl][:]],
                        outs=[tables[0][sl * rs:(sl + 1) * rs, :]])

            # ---- hops
            for t in range(NUM_HOP):
                tbl = tables[t]
                halves = (tbl[0:half, :], tbl[half:npad, :])
                ptiles = [None] * len(pieces)

                def emit_piece(pi, ptiles=ptiles, halves=halves, t=t):
                    if ptiles[pi] is not None:
                        return
                    b0, nblk, h = pieces[pi]
                    mt = mp.tile([P, nblk, D2], BF16, tag="msg",
                                 name=f"msg_t{t}_p{pi}")
                    nc.gpsimd.dma_gather(
                        mt[:], halves[h],
                        idx_all[:, b0 * 8:(b0 + nblk) * 8],
                        nblk * P, nblk * P, D2, single_packet=True,
                        queue_num=pi % NQUEUES)
                    ptiles[pi] = mt

                for c in range(cpc):
                    sts = []
                    for h in (0, 1):
                        b0 = (h * cpc + c) * K
                        st = slp.tile([P, K, D2], BF16, tag="sblk",
                                      name=f"sblk_t{t}_c{c}_h{h}")
                        nc.sync.dma_start(
                            st[:], sblk_d[:, b0 * D2:(b0 + K) * D2])
                        sts.append(st)
                        for k in range(K):
                            emit_piece(blk_piece[b0 + k][0])
                    ps = pp.tile([P, D2], F32, tag="agg", space="PSUM", bufs=3)
                    nmm = 2 * K
                    for ch in (0, 1):
                        mi = 0
                        for h in (0, 1):
                            b0 = (h * cpc + c) * K
                            for k in range(K):
                                pi, col = blk_piece[b0 + k]
                                mt = ptiles[pi]
                                nc.tensor.matmul(
                                    out=ps[:, ch * D:(ch + 1) * D],
                                    lhsT=sts[h][:, k, ch * D:(ch + 1) * D],
                                    rhs=mt[:, col, ch * D:(ch + 1) * D],
                                    start=(mi == 0),
                                    stop=(mi == nmm - 1))
                                mi += 1
                    tsb = wp.tile([P, D2], F32, tag="tsb")
                    nc.scalar.mul(tsb[:], ps[:], 1.0 - BETA)
                    nc.vector.tensor_tensor(out=hcur[:, c, 0:D],
                                            in0=tsb[:, 0:D],
                                            in1=h0b[:, c, :], op=ADD)
                    nc.vector.tensor_tensor(out=hcur[:, c, D:D2],
                                            in0=tsb[:, D:D2],
                                            in1=h0b[:, c, :], op=ADD)
                    if t < NUM_HOP - 1:
                        hp = wp.tile([P, D2], BF16, tag="hp")
                        nc.vector.tensor_scalar(
                            out=hp[:], in0=hcur[:, c, :],
                            scalar1=normc[:, c:c + 1], scalar2=None, op0=MUL)
                        sl, cl = divmod(c, cpg)
                        nc.sync.dma_start(
                            agin[(t + 1) % 2][sl][cl * P:(cl + 1) * P, :], hp[:])
                        if cl == cpg - 1:
                            nc.gpsimd.collective_compute(
                                "AllGather", AGOP,
                                replica_groups=[list(range(NCORES))],
                                ins=[agin[(t + 1) % 2][sl][:]],
                                outs=[tables[t + 1][sl * rs:(sl + 1) * rs, :]])

            # ---- final linear + relu
            for c in range(cpc):
                po = pp.tile([P, D2], F32, tag="pout", space="PSUM", bufs=2)
                for ch in (0, 1):
                    tp = pp.tile([P, P], BF16, tag="tps", space="PSUM", bufs=2)
                    nc.tensor.transpose(out=tp[:],
                                        in_=hcur[:, c, ch * D:(ch + 1) * D],
                                        identity=ident[:])
                    h4t = wp.tile([P, P], F32, tag="h4t")
                    nc.scalar.copy(h4t[:], tp[:])
                    nc.tensor.matmul(out=po[:, ch * D:(ch + 1) * D], lhsT=h4t[:],
                                     rhs=Wt[:, ch * D:(ch + 1) * D],
                                     start=True, stop=True)
                ob = wp.tile([P, D2], F32, tag="ob")
                nc.vector.tensor_tensor(out=ob[:], in0=po[:], in1=bt[:], op=ADD)
                ob2 = wp.tile([P, D2], F32, tag="ob2")
                nc.vector.tensor_scalar(out=ob2[:], in0=ob[:], scalar1=0.0,
                                        scalar2=None, op0=MAX)
                nc.sync.dma_start(out[c * P:(c + 1) * P, :], ob2[:])

    nc.compile()
    return nc


# --------------------------------------------------------------------------
# Entry point
# --------------------------------------------------------------------------

def run(features, src, dst, edge_factors, W, b, cpc=49, nsplit=7, trace=False):
    features = np.asarray(features, dtype=np.float32)
    src = np.asarray(src, dtype=np.int32)
    dst = np.asarray(dst, dtype=np.int32)
    edge_factors = np.asarray(edge_factors, dtype=np.float32)
    W = np.asarray(W, dtype=np.float32)
    b = np.asarray(b, dtype=np.float32)

    in_maps, struct, perm = _preprocess(features, src, dst, edge_factors, cpc, nsplit)
    W_in = np.concatenate([W[0], W[1]], axis=1).astype(np.float32)
    b_repl = np.tile(np.concatenate([b[0], b[1]])[None, :], (P, 1)).astype(np.float32)
    for im in in_maps:
        im["W_in"] = W_in
        im["b_repl"] = b_repl

    key = (struct["cpc"], struct["K"], struct["nsplit"])
    nc = _NC_CACHE.get(key)
    if nc is None:
        nc = _build(struct)
        _NC_CACHE[key] = nc

    res = run_bass_kernel_spmd(nc, in_maps, core_ids=list(range(NCORES)),
                               trace=trace)
    out_all = np.concatenate([res.results[m]["out"] for m in range(NCORES)], axis=0)
    result = out_all[perm]  # perm maps node -> slot
    return result.astype(np.float32), res


def kernel(**inputs):
    result, _ = run(**inputs)
    return result


# revision 13
# speedup vs baseline: 1.7523x; 1.2801x over previous
"""GCN 4-hop message passing on 8 Trainium2 NeuronCores.

Strategy:
  - Nodes are assigned to 128-wide "chunks" with degree-balanced packing (LPT);
    core m owns chunks [m*CPC, (m+1)*CPC). Edges are partitioned by destination
    chunk; within a (chunk, src-half) segment they are padded to a fixed
    number K of 128-edge blocks so the SPMD program is identical on all cores.
  - Per hop: each core dma_gathers source rows (channel-interleaved bf16 table
    in HBM, two halves for int16 indices; <=1024 idx per instruction so
    single_packet descriptor generation applies), loads host-precomputed
    one-hot-times-weight S blocks from HBM, and segment-sums via TensorEngine
    matmuls accumulated in PSUM. Node update (beta mix + norm) feeds an
    AllGather replicating the updated table to all cores.
  - Final per-graph Linear + ReLU via PE transpose + matmul.

Host-side work is limited to integer index/schedule construction, the static
one-hot weight blocks, and input/output reshuffling; all graph compute
(gather, message scaling, aggregation, update, linear) runs on device.
"""
import math

import numpy as np
import ml_dtypes

import concourse.bacc as bacc
import concourse.bass as bass
import concourse.mybir as mybir
import concourse.tile as tile
from concourse.bass_utils import run_bass_kernel_spmd

P = 128
NCORES = 8
G = 2
BETA = 0.1
NUM_HOP = 4
MAX_GATHER = 1024  # single_packet limit: 64 descs x 16 engines
NQUEUES = 4  # parallel SWDGE descriptor-generation queues

F32 = mybir.dt.float32
BF16 = mybir.dt.bfloat16
I16 = mybir.dt.int16

_NC_CACHE = {}


# --------------------------------------------------------------------------
# Host preprocessing
# --------------------------------------------------------------------------

def _lpt_pack(indeg, nchunk):
    """Assign nodes to nchunk chunks of P slots, balancing degree sums.

    Returns perm: node -> global slot id."""
    import heapq

    n = indeg.shape[0]
    order = np.argsort(-indeg, kind="stable")
    heap = [(0, c) for c in range(nchunk)]
    heapq.heapify(heap)
    counts = np.zeros(nchunk, dtype=np.int64)
    perm = np.empty(n, dtype=np.int64)
    deg = indeg.astype(np.int64)
    for v in order:
        s, c = heapq.heappop(heap)
        perm[v] = c * P + counts[c]
        counts[c] += 1
        if counts[c] < P:
            heapq.heappush(heap, (s + deg[v], c))
    return perm


def _preprocess(features, src, dst, edge_factors, cpc, nsplit):
    """Build per-core input arrays and the static schedule structure."""
    n, d = features.shape
    assert d == P
    assert cpc % nsplit == 0
    cpg = cpc // nsplit          # chunks per split region (per core)
    nchunk = NCORES * cpc
    npad = nchunk * P
    npc = cpc * P
    rs = NCORES * cpg * P        # rows per split region of the table
    half = npad // 2
    assert half <= 32768, f"half {half} exceeds int16 range"

    indeg = np.bincount(dst, minlength=n).astype(np.int64)
    norm = 1.0 / np.sqrt(np.clip(indeg, 1, None).astype(np.float64))
    perm = _lpt_pack(indeg, nchunk)

    # decompose LPT slot into (core m, position pos, lane i)
    cg = perm // P
    lane = perm % P
    m_of = cg // cpc
    pos_of = cg % cpc
    # table row (split-region-major, rank-major inside region: AllGather layout)
    perm_row = ((pos_of // cpg) * rs + m_of * (cpg * P)
                + (pos_of % cpg) * P + lane)
    # output index (core-major, position-major)
    perm_out = m_of * npc + pos_of * P + lane

    feat_slot = np.zeros((npad, d), dtype=np.float32)
    feat_slot[perm_out] = np.asarray(features, dtype=np.float32)
    norm_slot = np.ones(npad, dtype=np.float32)
    norm_slot[perm_out] = norm.astype(np.float32)

    e_m = m_of[dst]
    e_pos = pos_of[dst]
    e_dl = lane[dst]
    srow = perm_row[src]
    ef0 = np.asarray(edge_factors[0], dtype=np.float32)
    ef1 = np.asarray(edge_factors[1], dtype=np.float32)

    per_core = []
    kmax = 1
    for m in range(NCORES):
        sel = np.nonzero(e_m == m)[0]
        ch = e_pos[sel]
        dl = e_dl[sel].astype(np.int64)
        hf = srow[sel] // half
        sx = (srow[sel] % half).astype(np.int64)
        seg = hf * cpc + ch  # stream-major: half, then chunk position
        o2 = np.lexsort((sx, seg))
        seg, sx, dl = seg[o2], sx[o2], dl[o2]
        w0, w1 = ef0[sel][o2], ef1[sel][o2]
        cnt = np.bincount(seg, minlength=cpc * 2)
        kmax = max(kmax, int(math.ceil(cnt.max() / P)))
        per_core.append((seg, sx, dl, w0, w1, cnt))

    K = kmax
    btot = cpc * 2 * K
    # block id of (c, h, k) = (h*cpc + c)*K + k
    # gather instruction pieces: within each half-stream, runs of <= 8 blocks
    blocks_per_half = cpc * K
    pieces = []  # (block0, nblk, half)
    maxb = MAX_GATHER // P
    for h in (0, 1):
        b = h * blocks_per_half
        end = (h + 1) * blocks_per_half
        while b < end:
            nb = min(maxb, end - b)
            pieces.append((b, nb, h))
            b += nb

    in_maps = []
    ident = np.eye(P, dtype=ml_dtypes.bfloat16)

    for m in range(NCORES):
        seg, sx, dl, w0, w1, cnt = per_core[m]
        starts = np.zeros(cpc * 2, dtype=np.int64)
        starts[1:] = np.cumsum(cnt)[:-1]

        s_idx = np.zeros(btot * P, dtype=np.int64)
        s_dl = np.zeros(btot * P, dtype=np.int64)
        s_w0 = np.zeros(btot * P, dtype=np.float32)
        s_w1 = np.zeros(btot * P, dtype=np.float32)
        w0 = w0 * (1.0 - BETA)
        w1 = w1 * (1.0 - BETA)
        for s in range(cpc * 2):
            # seg s = hf*cpc + ch maps to block base s*K
            c0 = s * K * P
            k = int(cnt[s])
            st = starts[s]
            s_idx[c0:c0 + k] = sx[st:st + k]
            s_dl[c0:c0 + k] = dl[st:st + k]
            s_w0[c0:c0 + k] = w0[st:st + k]
            s_w1[c0:c0 + k] = w1[st:st + k]

        idx_all = np.zeros((128, btot * 8), dtype=np.int16)
        for (b0, nblk, _h) in pieces:
            v = s_idx[b0 * P:(b0 + nblk) * P].astype(np.int16)
            idx_all[:16, b0 * 8:(b0 + nblk) * 8] = v.reshape(nblk * 8, 16).T
        idx_all[16:] = np.tile(idx_all[:16], (7, 1))

        # chunk-major block order for the DVE S-build: [c][h][k]
        # gather-stream block id (h*cpc + c)*K + k -> chunk-major c*2K + h*K + k
        cm = np.arange(btot)
        hh = cm // (cpc * K)
        rest = cm % (cpc * K)
        cc_ = rest // K
        kk = rest % K
        cmaj = cc_ * (2 * K) + hh * K + kk  # stream block -> chunk-major col
        dl2 = np.zeros((128, btot), dtype=ml_dtypes.bfloat16)
        wa2 = np.zeros((128, btot), dtype=ml_dtypes.bfloat16)
        wb2 = np.zeros((128, btot), dtype=ml_dtypes.bfloat16)
        dl2[:, cmaj] = s_dl.reshape(btot, P).T
        wa2[:, cmaj] = s_w0.reshape(btot, P).T.astype(ml_dtypes.bfloat16)
        wb2[:, cmaj] = s_w1.reshape(btot, P).T.astype(ml_dtypes.bfloat16)

        in_maps.append({
            "feat": feat_slot[m * npc:(m + 1) * npc],
            "normc": norm_slot[m * npc:(m + 1) * npc].reshape(cpc, P).T.copy(),
            "idx_all": idx_all,
            "dstloc": dl2,
            "wa": wa2,
            "wb": wb2,
            "iota": np.tile(np.arange(P, dtype=ml_dtypes.bfloat16), (P, 1)),
            "ident": ident,
        })

    struct = dict(cpc=cpc, K=K, pieces=pieces, nsplit=nsplit, cpg=cpg, rs=rs,
                  npad=npad, npc=npc, half=half)
    return in_maps, struct, perm_out


# --------------------------------------------------------------------------
# Bass program
# --------------------------------------------------------------------------

def _build(struct):
    cpc = struct["cpc"]
    K = struct["K"]
    pieces = struct["pieces"]
    npad = struct["npad"]
    npc = struct["npc"]
    half = struct["half"]
    nsplit = struct["nsplit"]
    cpg = struct["cpg"]
    rs = struct["rs"]
    D = P
    D2 = 2 * P
    btot = cpc * 2 * K

    # block id -> (piece index, col within piece)
    blk_piece = {}
    for pi, (b0, nblk, _h) in enumerate(pieces):
        for j in range(nblk):
            blk_piece[b0 + j] = (pi, j)

    nc = bacc.Bacc("TRN2", target_bir_lowering=False, debug=False,
                   enable_asserts=False, num_devices=NCORES,
                   num_swdge_queues=NQUEUES)

    feat = nc.dram_tensor("feat", [npc, D], F32, kind="ExternalInput").ap()
    normc_d = nc.dram_tensor("normc", [P, cpc], F32, kind="ExternalInput").ap()
    idx_d = nc.dram_tensor("idx_all", [128, btot * 8], I16, kind="ExternalInput").ap()
    dstloc_d = nc.dram_tensor("dstloc", [128, btot], BF16, kind="ExternalInput").ap()
    wa_d = nc.dram_tensor("wa", [128, btot], BF16, kind="ExternalInput").ap()
    wb_d = nc.dram_tensor("wb", [128, btot], BF16, kind="ExternalInput").ap()
    iota_d = nc.dram_tensor("iota", [P, P], BF16, kind="ExternalInput").ap()
    ident_d = nc.dram_tensor("ident", [P, P], BF16, kind="ExternalInput").ap()
    W_d = nc.dram_tensor("W_in", [P, D2], F32, kind="ExternalInput").ap()
    b_d = nc.dram_tensor("b_repl", [P, D2], F32, kind="ExternalInput").ap()
    out = nc.dram_tensor("out", [npc, D2], F32, kind="ExternalOutput").ap()

    AGOP = mybir.AluOpType.bypass
    ADD = mybir.AluOpType.add
    MUL = mybir.AluOpType.mult
    MAX = mybir.AluOpType.max

    with tile.TileContext(nc) as tc:
        with (
            tc.tile_pool(name="const", bufs=1) as cp,
            tc.tile_pool(name="state", bufs=1) as sp,
            tc.tile_pool(name="msg", bufs=10) as mp,
            tc.tile_pool(name="sload", bufs=4) as slp,
            tc.tile_pool(name="work", bufs=3) as wp,
            tc.tile_pool(name="psum", bufs=4, space="PSUM") as pp,
            tc.tile_pool(name="dram", bufs=1, space="DRAM") as dp,
        ):
            normc = cp.tile([P, cpc], F32, tag="normc")
            idx_all = cp.tile([128, btot * 8], I16, tag="idx")
            ident = cp.tile([P, P], BF16, tag="ident")
            iota = cp.tile([P, P], BF16, tag="iota")
            dstloc = cp.tile([128, btot], BF16, tag="dstloc")
            wat = cp.tile([128, btot], BF16, tag="wa")
            wbt = cp.tile([128, btot], BF16, tag="wb")
            Wt = cp.tile([P, D2], F32, tag="W")
            bt = cp.tile([P, D2], F32, tag="b")
            h0b = sp.tile([P, cpc, D2], BF16, tag="h0b")
            hcur = sp.tile([P, cpc, D2], BF16, tag="hcur")

            for t_, d_ in ((normc, normc_d), (idx_all, idx_d),
                           (ident, ident_d), (iota, iota_d),
                           (dstloc, dstloc_d), (wat, wa_d), (wbt, wb_d),
                           (Wt, W_d), (bt, b_d)):
                nc.sync.dma_start(t_[:], d_[:])

            tables = [dp.tile([npad, D2], BF16, tag=f"table{t}", name=f"table{t}")
                      for t in range(NUM_HOP)]
            agin = [[dp.tile([cpg * P, D2], BF16, tag=f"agin{i}_{sl}",
                             name=f"agin{i}_{sl}") for sl in range(nsplit)]
                    for i in range(2)]

            # ---- prologue: table0 = (features * norm) duplicated per channel
            for c in range(cpc):
                ft = wp.tile([P, D], F32, tag="ft")
                nc.sync.dma_start(ft[:], feat[c * P:(c + 1) * P, :])
                nc.vector.tensor_scalar(out=h0b[:, c, 0:D], in0=ft[:],
                                        scalar1=BETA, scalar2=None, op0=MUL)
                nc.vector.tensor_scalar(out=h0b[:, c, D:D2], in0=ft[:],
                                        scalar1=BETA, scalar2=None, op0=MUL)
                hp = wp.tile([P, D2], BF16, tag="hp")
                nc.vector.tensor_scalar(out=hp[:, 0:D], in0=ft[:],
                                        scalar1=normc[:, c:c + 1], scalar2=None,
                                        op0=MUL)
                nc.vector.tensor_scalar(out=hp[:, D:D2], in0=ft[:],
                                        scalar1=normc[:, c:c + 1], scalar2=None,
                                        op0=MUL)
                sl, cl = divmod(c, cpg)
                nc.sync.dma_start(agin[0][sl][cl * P:(cl + 1) * P, :], hp[:])
                if cl == cpg - 1:
                    nc.gpsimd.collective_compute(
                        "AllGather", AGOP,
                        replica_groups=[list(range(NCORES))],
                        ins=[agin[0][sl][:]],
                        outs=[tables[0][sl * rs:(sl + 1) * rs, :]])

            # ---- hops
            for t in range(NUM_HOP):
                tbl = tables[t]
                halves = (tbl[0:half, :], tbl[half:npad, :])
                ptiles = [None] * len(pieces)

                def emit_piece(pi, ptiles=ptiles, halves=halves, t=t):
                    if ptiles[pi] is not None:
                        return
                    b0, nblk, h = pieces[pi]
                    mt = mp.tile([P, nblk, D2], BF16, tag="msg",
                                 name=f"msg_t{t}_p{pi}")
                    nc.gpsimd.dma_gather(
                        mt[:], halves[h],
                        idx_all[:, b0 * 8:(b0 + nblk) * 8],
                        nblk * P, nblk * P, D2, single_packet=True,
                        queue_num=pi % NQUEUES)
                    ptiles[pi] = mt

                K2 = 2 * K
                ISEQ = mybir.AluOpType.is_equal
                for c in range(cpc):
                    for h in (0, 1):
                        b0 = (h * cpc + c) * K
                        for k in range(K):
                            emit_piece(blk_piece[b0 + k][0])
                    # build S for this chunk on DVE (chunk-major cols)
                    c0 = c * K2
                    dcol = dstloc[:, c0:c0 + K2, None].to_broadcast([P, K2, D])
                    iob = iota[:, None, :].to_broadcast([P, K2, D])
                    msk = slp.tile([P, K2, D], BF16, tag="msk", bufs=2,
                                   name=f"msk_t{t}_c{c}")
                    nc.vector.tensor_tensor(out=msk[:], in0=iob, in1=dcol,
                                            op=ISEQ)
                    S0 = slp.tile([P, K2, D], BF16, tag="S0", bufs=3,
                                  name=f"S0_t{t}_c{c}")
                    S1 = slp.tile([P, K2, D], BF16, tag="S1", bufs=3,
                                  name=f"S1_t{t}_c{c}")
                    wac = wat[:, c0:c0 + K2, None].to_broadcast([P, K2, D])
                    wbc = wbt[:, c0:c0 + K2, None].to_broadcast([P, K2, D])
                    nc.vector.tensor_tensor(out=S0[:], in0=msk[:], in1=wac,
                                            op=MUL)
                    nc.vector.tensor_tensor(out=S1[:], in0=msk[:], in1=wbc,
                                            op=MUL)
                    Ss = (S0, S1)
                    ps = pp.tile([P, D2], F32, tag="agg", space="PSUM", bufs=3)
                    nmm = 2 * K
                    for ch in (0, 1):
                        mi = 0
                        for h in (0, 1):
                            b0 = (h * cpc + c) * K
                            for k in range(K):
                                pi, col = blk_piece[b0 + k]
                                mt = ptiles[pi]
                                nc.tensor.matmul(
                                    out=ps[:, ch * D:(ch + 1) * D],
                                    lhsT=Ss[ch][:, h * K + k, :],
                                    rhs=mt[:, col, ch * D:(ch + 1) * D],
                                    start=(mi == 0),
                                    stop=(mi == nmm - 1))
                                mi += 1
                    nc.vector.tensor_tensor(out=hcur[:, c, :],
                                            in0=ps[:],
                                            in1=h0b[:, c, :], op=ADD)
                    if t < NUM_HOP - 1:
                        hp = wp.tile([P, D2], BF16, tag="hp")
                        nc.vector.tensor_scalar(
                            out=hp[:], in0=hcur[:, c, :],
                            scalar1=normc[:, c:c + 1], scalar2=None, op0=MUL)
                        sl, cl = divmod(c, cpg)
                        nc.sync.dma_start(
                            agin[(t + 1) % 2][sl][cl * P:(cl + 1) * P, :], hp[:])
                        if cl == cpg - 1:
                            nc.gpsimd.collective_compute(
                                "AllGather", AGOP,
                                replica_groups=[list(range(NCORES))],
                                ins=[agin[(t + 1) % 2][sl][:]],
                                outs=[tables[t + 1][sl * rs:(sl + 1) * rs, :]])

            # ---- final linear + relu
            for c in range(cpc):
                po = pp.tile([P, D2], F32, tag="pout", space="PSUM", bufs=2)
                for ch in (0, 1):
                    tp = pp.tile([P, P], BF16, tag="tps", space="PSUM", bufs=2)
                    nc.tensor.transpose(out=tp[:],
                                        in_=hcur[:, c, ch * D:(ch + 1) * D],
                                        identity=ident[:])
                    h4t = wp.tile([P, P], F32, tag="h4t")
                    nc.scalar.copy(h4t[:], tp[:])
                    nc.tensor.matmul(out=po[:, ch * D:(ch + 1) * D], lhsT=h4t[:],
                                     rhs=Wt[:, ch * D:(ch + 1) * D],
                                     start=True, stop=True)
                ob = wp.tile([P, D2], F32, tag="ob")
                nc.vector.tensor_tensor(out=ob[:], in0=po[:], in1=bt[:], op=ADD)
                ob2 = wp.tile([P, D2], F32, tag="ob2")
                nc.vector.tensor_scalar(out=ob2[:], in0=ob[:], scalar1=0.0,
                                        scalar2=None, op0=MAX)
                nc.sync.dma_start(out[c * P:(c + 1) * P, :], ob2[:])

    nc.compile()
    return nc


# --------------------------------------------------------------------------
# Entry point
# --------------------------------------------------------------------------

def run(features, src, dst, edge_factors, W, b, cpc=49, nsplit=7, trace=False):
    features = np.asarray(features, dtype=np.float32)
    src = np.asarray(src, dtype=np.int32)
    dst = np.asarray(dst, dtype=np.int32)
    edge_factors = np.asarray(edge_factors, dtype=np.float32)
    W = np.asarray(W, dtype=np.float32)
    b = np.asarray(b, dtype=np.float32)

    in_maps, struct, perm = _preprocess(features, src, dst, edge_factors, cpc, nsplit)
    W_in = np.concatenate([W[0], W[1]], axis=1).astype(np.float32)
    b_repl = np.tile(np.concatenate([b[0], b[1]])[None, :], (P, 1)).astype(np.float32)
    for im in in_maps:
        im["W_in"] = W_in
        im["b_repl"] = b_repl

    key = (struct["cpc"], struct["K"], struct["nsplit"])
    nc = _NC_CACHE.get(key)
    if nc is None:
        nc = _build(struct)
        _NC_CACHE[key] = nc

    res = run_bass_kernel_spmd(nc, in_maps, core_ids=list(range(NCORES)),
                               trace=trace)
    out_all = np.concatenate([res.results[m]["out"] for m in range(NCORES)], axis=0)
    result = out_all[perm]  # perm maps node -> slot
    return result.astype(np.float32), res


def kernel(**inputs):
    result, _ = run(**inputs)
    return result
